# revision 3
# baseline (speedup 1.0000x reference)
"""GEMM + reduce-scatter (nn_GemmRSIntraNode) as a Bass/Tile kernel on 8 trn2 cores.

Full semantics: out = einsum('rmk,rnk->mn', input, weight).reshape(8, 1024, 4096)
with input [8, 8192, 1024] f32 and weight [8, 4096, 1024] f32.

Sharding choice: each core c directly computes output rows
[c*1024:(c+1)*1024] of the reduced result — a [1024, 8192] x [8192, 4096]
GEMM per core where the contraction axis is (rank, local_k) flattened.
The rank-sum IS the K-axis contraction, so no cross-core communication is
needed; the "reduce-scatter" is absorbed into the GEMM. Inputs are
pre-transposed host-side into [K, M] / [K, N] layouts (bf16) so the
device kernel does only contiguous DMA and matmuls.

Perf structure (v2): IO in bf16 (norm_rel ~1.5e-3, gate is 2e-2). The
whole per-core A panel [K, 1024] bf16 = 16 MB stays resident in SBUF, so
B [K, 4096] bf16 = 64 MB streams through exactly once. DMAs are packed
KPACK k-chunks at a time (1-2 MB per dma_start) to amortize the ~2 us
per-DMA completion latency, and B loads alternate between the two HWDGE
rings (sync / scalar) while A loads ride the SWDGE (gpsimd) ring.
Output stores are batched one per n-block ([128, 8, 512] DRAM AP).
"""

import os
from contextlib import ExitStack

import numpy as np

WS = 8
M = 8192
N = 4096
LK = 1024
K = WS * LK          # 8192 contraction (rank*local_k)
M_LOC = M // WS      # 1024 output rows per core
N_CORES = 8

DTYPE = os.environ.get("BASS_KERNEL_DTYPE", "bfloat16")

_NC_CACHE = {}


def _build_nc(dt_name):
    import concourse.tile as tile
    from concourse import bacc, mybir

    f32 = mybir.dt.float32
    if dt_name == "bfloat16":
        io_dt = mybir.dt.bfloat16
    elif dt_name == "float32r":
        io_dt = mybir.dt.float32r
    else:
        raise ValueError(dt_name)

    KC = K // 128                                  # 64 k-chunks of 128
    KPACK = int(os.environ.get("BASS_KPACK", "8"))  # k-chunks per DMA
    KG = KC // KPACK                               # dma groups along K
    NB = N // 512                                  # 8 n-blocks
    MSUB = M_LOC // 128                            # 8 psum row-tiles
    B_BUFS = int(os.environ.get("BASS_B_BUFS", "4"))
    C_BUFS = int(os.environ.get("BASS_C_BUFS", "2"))
    SPLIT = os.environ.get("BASS_SPLIT", "1") == "1"   # B loads on 2 rings
    A_ENGINE = os.environ.get("BASS_A_ENGINE", "gpsimd")
    REPS = int(os.environ.get("BASS_REPS", "1"))
    assert MSUB <= 8                               # one PSUM bank per row-tile

    nc = bacc.Bacc("TRN2", target_bir_lowering=False, debug=False,
                   num_devices=N_CORES)
    a_d = nc.dram_tensor("a", [K, M_LOC], io_dt, kind="ExternalInput")
    b_d = nc.dram_tensor("b", [K, N], io_dt, kind="ExternalInput")
    o_d = nc.dram_tensor("o", [M_LOC, N], f32, kind="ExternalOutput")

    with tile.TileContext(nc) as tc, ExitStack() as ctx:
        apool = ctx.enter_context(tc.tile_pool(name="apool", bufs=KG))
        bpool = ctx.enter_context(tc.tile_pool(name="bpool", bufs=B_BUFS))
        cpool = ctx.enter_context(tc.tile_pool(name="cpool", bufs=C_BUFS))
        pp = ctx.enter_context(tc.tile_pool(name="pp", bufs=8, space="PSUM"))

        a_eng = getattr(nc, A_ENGINE)

        def load_a(g):
            # [128, KPACK, M_LOC] : partition p holds k-rows g*KPACK*128
            # + j*128 + p for j in 0..KPACK
            a_t = apool.tile([128, KPACK * M_LOC], io_dt, name=f"a_{g}",
                             tag="a")
            src = a_d.ap()[g * KPACK * 128:(g + 1) * KPACK * 128, :] \
                .rearrange("(j p) m -> p j m", p=128)
            a_eng.dma_start(a_t[:], src)
            return a_t

        a_tiles = [load_a(g) for g in range(KG)]

        for rep in range(REPS):
            for nb in range(NB):
                psums = []
                for ms in range(MSUB):
                    p_t = pp.tile([128, 512], f32, name=f"p_{nb}_{ms}",
                                  tag="p")
                    psums.append(p_t)
                for g in range(KG):
                    b_t = bpool.tile([128, KPACK * 512], io_dt,
                                     name=f"b_{nb}_{g}", tag="b")
                    src = b_d.ap()[g * KPACK * 128:(g + 1) * KPACK * 128,
                                   nb * 512:(nb + 1) * 512] \
                        .rearrange("(j p) n -> p j n", p=128)
                    eng = nc.scalar if (SPLIT and g % 2) else nc.sync
                    eng.dma_start(b_t[:], src)
                    for j in range(KPACK):
                        kc = g * KPACK + j
                        rhs = b_t[:, j * 512:(j + 1) * 512]
                        for ms in range(MSUB):
                            lhsT = a_tiles[g][:, j * M_LOC + ms * 128:
                                              j * M_LOC + (ms + 1) * 128]
                            nc.tensor.matmul(psums[ms][:], lhsT, rhs,
                                             start=(kc == 0),
                                             stop=(kc == KC - 1))
                c_t = cpool.tile([128, MSUB * 512], f32, name=f"c_{nb}",
                                 tag="c")
                for ms in range(MSUB):
                    nc.vector.tensor_copy(c_t[:, ms * 512:(ms + 1) * 512],
                                          psums[ms][:])
                dst = o_d.ap()[:, nb * 512:(nb + 1) * 512] \
                    .rearrange("(ms p) n -> p ms n", p=128)
                nc.sync.dma_start(dst, c_t[:])

    nc.compile()
    return nc


def get_nc(dt_name=None):
    dt_name = dt_name or DTYPE
    if dt_name not in _NC_CACHE:
        _NC_CACHE[dt_name] = _build_nc(dt_name)
    return _NC_CACHE[dt_name]


def make_in_maps(input, weight, dt_name=None):
    """Host-side shard + layout prep. Returns in_maps for cores 0..7."""
    dt_name = dt_name or DTYPE
    input = np.asarray(input, dtype=np.float32)
    weight = np.asarray(weight, dtype=np.float32)
    assert input.shape == (WS, M, LK), input.shape
    assert weight.shape == (WS, N, LK), weight.shape

    if dt_name == "bfloat16":
        import ml_dtypes
        np_dt = ml_dtypes.bfloat16
    else:
        np_dt = np.float32

    # B[r*LK + k, n] = weight[r, n, k]  -> [K, N]
    b_full = np.ascontiguousarray(
        weight.transpose(0, 2, 1).reshape(K, N).astype(np_dt))
    in_maps = []
    for c in range(N_CORES):
        # A_c[r*LK + k, m] = input[r, c*M_LOC + m, k]  -> [K, M_LOC]
        a_c = np.ascontiguousarray(
            input[:, c * M_LOC:(c + 1) * M_LOC, :]
            .transpose(0, 2, 1).reshape(K, M_LOC).astype(np_dt))
        in_maps.append({"a": a_c, "b": b_full})
    return in_maps


def kernel(input, weight):
    from concourse import bass_utils

    nc = get_nc()
    in_maps = make_in_maps(input, weight)
    res = bass_utils.run_bass_kernel_spmd(
        nc, in_maps, core_ids=list(range(N_CORES)))
    out = np.stack([res.results[c]["o"] for c in range(N_CORES)], axis=0)
    return out.astype(np.float32)


# revision 6
# speedup vs baseline: 2.4175x; 2.4175x over previous
"""GEMM + reduce-scatter (nn_GemmRSIntraNode) as a Bass/Tile kernel on 8 trn2 cores.

Full semantics: out = einsum('rmk,rnk->mn', input, weight).reshape(8, 1024, 4096)
with input [8, 8192, 1024] f32 and weight [8, 4096, 1024] f32.

Sharding choice: each core c directly computes output rows
[c*1024:(c+1)*1024] of the reduced result — a [1024, 8192] x [8192, 4096]
GEMM per core where the contraction axis is (rank, local_k) flattened.
The rank-sum IS the K-axis contraction, so no cross-core communication is
needed; the "reduce-scatter" is absorbed into the GEMM. Inputs are
pre-transposed host-side into [K, M] / [K, N] layouts (bf16) so the
device kernel does only contiguous DMA and matmuls.

Perf structure (v2): IO in bf16 (norm_rel ~1.5e-3, gate is 2e-2). The
whole per-core A panel [K, 1024] bf16 = 16 MB stays resident in SBUF, so
B [K, 4096] bf16 = 64 MB streams through exactly once. DMAs are packed
KPACK k-chunks at a time (1-2 MB per dma_start) to amortize the ~2 us
per-DMA completion latency, and B loads alternate between the two HWDGE
rings (sync / scalar) while A loads ride the SWDGE (gpsimd) ring.
Output stores are batched one per n-block ([128, 8, 512] DRAM AP).
"""

import os
from contextlib import ExitStack

import numpy as np

WS = 8
M = 8192
N = 4096
LK = 1024
K = WS * LK          # 8192 contraction (rank*local_k)
M_LOC = M // WS      # 1024 output rows per core
N_CORES = 8

DTYPE = os.environ.get("BASS_KERNEL_DTYPE", "bfloat16")

_NC_CACHE = {}


def _build_nc(dt_name):
    import concourse.tile as tile
    from concourse import bacc, mybir

    f32 = mybir.dt.float32
    if dt_name == "bfloat16":
        io_dt = mybir.dt.bfloat16
    elif dt_name == "float32r":
        io_dt = mybir.dt.float32r
    else:
        raise ValueError(dt_name)

    KC = K // 128                                  # 64 k-chunks of 128
    KPACK = int(os.environ.get("BASS_KPACK", "8"))  # k-chunks per DMA
    KG = KC // KPACK                               # dma groups along K
    NB = N // 512                                  # 8 n-blocks
    MSUB = M_LOC // 128                            # 8 psum row-tiles
    B_BUFS = int(os.environ.get("BASS_B_BUFS", "4"))
    C_BUFS = int(os.environ.get("BASS_C_BUFS", "2"))
    SPLIT = os.environ.get("BASS_SPLIT", "0") == "1"   # B loads on 2 rings
    A_ENGINE = os.environ.get("BASS_A_ENGINE", "gpsimd")
    # drain: "both" splits psum->sbuf copies across DVE+ACT so the bank the
    # next n-block's first matmuls need frees ~2x sooner; "dve" uses DVE only
    DRAIN = os.environ.get("BASS_DRAIN", "both")
    OUT_BF16 = os.environ.get("BASS_OUT_BF16", "1") == "1"
    REPS = int(os.environ.get("BASS_REPS", "1"))
    assert MSUB <= 8                               # one PSUM bank per row-tile

    nc = bacc.Bacc("TRN2", target_bir_lowering=False, debug=False,
                   num_devices=N_CORES)
    out_dt = mybir.dt.bfloat16 if OUT_BF16 else f32
    a_d = nc.dram_tensor("a", [K, M_LOC], io_dt, kind="ExternalInput")
    b_d = nc.dram_tensor("b", [K, N], io_dt, kind="ExternalInput")
    o_d = nc.dram_tensor("o", [M_LOC, N], out_dt, kind="ExternalOutput")

    with tile.TileContext(nc) as tc, ExitStack() as ctx:
        apool = ctx.enter_context(tc.tile_pool(name="apool", bufs=KG))
        bpool = ctx.enter_context(tc.tile_pool(name="bpool", bufs=B_BUFS))
        cpool = ctx.enter_context(tc.tile_pool(name="cpool", bufs=C_BUFS))
        pp = ctx.enter_context(tc.tile_pool(name="pp", bufs=8, space="PSUM"))

        a_eng = getattr(nc, A_ENGINE)

        def load_a(g):
            # [128, KPACK, M_LOC] : partition p holds k-rows g*KPACK*128
            # + j*128 + p for j in 0..KPACK
            a_t = apool.tile([128, KPACK * M_LOC], io_dt, name=f"a_{g}",
                             tag="a")
            src = a_d.ap()[g * KPACK * 128:(g + 1) * KPACK * 128, :] \
                .rearrange("(j p) m -> p j m", p=128)
            a_eng.dma_start(a_t[:], src)
            return a_t

        a_tiles = [load_a(g) for g in range(KG)]

        for rep in range(REPS):
            for nb in range(NB):
                psums = []
                for ms in range(MSUB):
                    p_t = pp.tile([128, 512], f32, name=f"p_{nb}_{ms}",
                                  tag="p")
                    psums.append(p_t)
                for g in range(KG):
                    b_t = bpool.tile([128, KPACK * 512], io_dt,
                                     name=f"b_{nb}_{g}", tag="b")
                    src = b_d.ap()[g * KPACK * 128:(g + 1) * KPACK * 128,
                                   nb * 512:(nb + 1) * 512] \
                        .rearrange("(j p) n -> p j n", p=128)
                    eng = nc.scalar if (SPLIT and g % 2) else nc.sync
                    eng.dma_start(b_t[:], src)
                    for j in range(KPACK):
                        kc = g * KPACK + j
                        rhs = b_t[:, j * 512:(j + 1) * 512]
                        for ms in range(MSUB):
                            lhsT = a_tiles[g][:, j * M_LOC + ms * 128:
                                              j * M_LOC + (ms + 1) * 128]
                            nc.tensor.matmul(psums[ms][:], lhsT, rhs,
                                             start=(kc == 0),
                                             stop=(kc == KC - 1))
                c_t = cpool.tile([128, MSUB * 512], out_dt, name=f"c_{nb}",
                                 tag="c")
                for ms in range(MSUB):
                    dst_sl = c_t[:, ms * 512:(ms + 1) * 512]
                    if DRAIN == "both" and ms % 2:
                        nc.scalar.activation(
                            dst_sl, psums[ms][:],
                            mybir.ActivationFunctionType.Copy)
                    else:
                        nc.vector.tensor_copy(dst_sl, psums[ms][:])
                dst = o_d.ap()[:, nb * 512:(nb + 1) * 512] \
                    .rearrange("(ms p) n -> p ms n", p=128)
                nc.sync.dma_start(dst, c_t[:])

    nc.compile()
    return nc


def get_nc(dt_name=None):
    dt_name = dt_name or DTYPE
    if dt_name not in _NC_CACHE:
        _NC_CACHE[dt_name] = _build_nc(dt_name)
    return _NC_CACHE[dt_name]


def make_in_maps(input, weight, dt_name=None):
    """Host-side shard + layout prep. Returns in_maps for cores 0..7."""
    dt_name = dt_name or DTYPE
    input = np.asarray(input, dtype=np.float32)
    weight = np.asarray(weight, dtype=np.float32)
    assert input.shape == (WS, M, LK), input.shape
    assert weight.shape == (WS, N, LK), weight.shape

    if dt_name == "bfloat16":
        import ml_dtypes
        np_dt = ml_dtypes.bfloat16
    else:
        np_dt = np.float32

    # B[r*LK + k, n] = weight[r, n, k]  -> [K, N]
    b_full = np.ascontiguousarray(
        weight.transpose(0, 2, 1).reshape(K, N).astype(np_dt))
    in_maps = []
    for c in range(N_CORES):
        # A_c[r*LK + k, m] = input[r, c*M_LOC + m, k]  -> [K, M_LOC]
        a_c = np.ascontiguousarray(
            input[:, c * M_LOC:(c + 1) * M_LOC, :]
            .transpose(0, 2, 1).reshape(K, M_LOC).astype(np_dt))
        in_maps.append({"a": a_c, "b": b_full})
    return in_maps


def kernel(input, weight):
    from concourse import bass_utils

    nc = get_nc()
    in_maps = make_in_maps(input, weight)
    res = bass_utils.run_bass_kernel_spmd(
        nc, in_maps, core_ids=list(range(N_CORES)))
    out = np.stack([res.results[c]["o"] for c in range(N_CORES)], axis=0)
    return out.astype(np.float32)


# revision 15
# speedup vs baseline: 4.6364x; 1.9179x over previous
"""GEMM + reduce-scatter (nn_GemmRSIntraNode) as a Bass/Tile kernel on 8 trn2 cores.

Full semantics: out = einsum('rmk,rnk->mn', input, weight).reshape(8, 1024, 4096)
with input [8, 8192, 1024] f32 and weight [8, 4096, 1024] f32.

Sharding choice: each core c directly computes output rows
[c*1024:(c+1)*1024] of the reduced result — a [1024, 8192] x [8192, 4096]
GEMM per core where the contraction axis is (rank, local_k) flattened.
The rank-sum IS the K-axis contraction, so no cross-core communication is
needed; the "reduce-scatter" is absorbed into the GEMM. Inputs are
pre-transposed host-side into [K, M] / [K, N] layouts (bf16) so the
device kernel does only contiguous DMA and matmuls.

Perf structure (v2): IO in bf16 (norm_rel ~1.5e-3, gate is 2e-2). The
whole per-core A panel [K, 1024] bf16 = 16 MB stays resident in SBUF, so
B [K, 4096] bf16 = 64 MB streams through exactly once. DMAs are packed
KPACK k-chunks at a time (1-2 MB per dma_start) to amortize the ~2 us
per-DMA completion latency, and B loads alternate between the two HWDGE
rings (sync / scalar) while A loads ride the SWDGE (gpsimd) ring.
Output stores are batched one per n-block ([128, 8, 512] DRAM AP).
"""

import os
from contextlib import ExitStack

import numpy as np

WS = 8
M = 8192
N = 4096
LK = 1024
K = WS * LK          # 8192 contraction (rank*local_k)
M_LOC = M // WS      # 1024 output rows per core
N_CORES = 8

DTYPE = os.environ.get("BASS_KERNEL_DTYPE", "bfloat16")

_NC_CACHE = {}


def _build_nc(dt_name):
    import concourse.tile as tile
    from concourse import bacc, mybir

    f32 = mybir.dt.float32
    if dt_name == "bfloat16":
        io_dt = mybir.dt.bfloat16
    elif dt_name == "float32r":
        io_dt = mybir.dt.float32r
    else:
        raise ValueError(dt_name)

    KC = K // 128                                  # 64 k-chunks of 128
    # NSUB: n-subtiles sharing one stationary operand (LDWEIGHTS amortization)
    NSUB = int(os.environ.get("BASS_NSUB", "1"))
    NBW = 512 * NSUB                               # n-block width
    NBI = N // NBW                                 # n-blocks
    MSP = (M_LOC // 128) // NSUB                   # m-subtiles per pass
    KPACK = int(os.environ.get("BASS_KPACK", str(8 // NSUB)))
    KG = KC // KPACK                               # dma groups along K
    NB = N // 512                                  # 8 n-blocks
    MSUB = M_LOC // 128                            # 8 psum row-tiles
    B_BUFS = int(os.environ.get("BASS_B_BUFS", "4"))
    C_BUFS = int(os.environ.get("BASS_C_BUFS", "2"))
    SPLIT = os.environ.get("BASS_SPLIT", "0") == "1"   # B loads on 2 rings
    A_ENGINE = os.environ.get("BASS_A_ENGINE", "gpsimd")
    # drain: "both" splits psum->sbuf copies across DVE+ACT so the bank the
    # next n-block's first matmuls need frees ~2x sooner; "dve" uses DVE only
    DRAIN = os.environ.get("BASS_DRAIN", "both")
    OUT_BF16 = os.environ.get("BASS_OUT_BF16", "1") == "1"
    # engine for output stores; "scalar" keeps the sync ring pure B-loads so
    # the store's wait-for-drain never blocks issuing the next block's loads
    STORE_ENGINE = os.environ.get("BASS_STORE_ENGINE", "scalar")
    # surgical-isolation knobs (experiments only; wrong results when set)
    NO_DRAIN = os.environ.get("BASS_NO_DRAIN", "0") == "1"
    FAKE_B = os.environ.get("BASS_FAKE_B", "0") == "1"
    REPS = int(os.environ.get("BASS_REPS", "1"))
    assert MSUB <= 8                               # one PSUM bank per row-tile

    nc = bacc.Bacc("TRN2", target_bir_lowering=False, debug=False,
                   num_devices=N_CORES)
    out_dt = mybir.dt.bfloat16 if OUT_BF16 else f32
    a_d = nc.dram_tensor("a", [K, M_LOC], io_dt, kind="ExternalInput")
    b_d = nc.dram_tensor("b", [K, N], io_dt, kind="ExternalInput")
    o_d = nc.dram_tensor("o", [M_LOC, N], out_dt, kind="ExternalOutput")

    with tile.TileContext(nc) as tc, ExitStack() as ctx:
        apool = ctx.enter_context(tc.tile_pool(name="apool", bufs=KC // 8))
        bpool = ctx.enter_context(tc.tile_pool(name="bpool", bufs=B_BUFS))
        cpool = ctx.enter_context(tc.tile_pool(name="cpool", bufs=C_BUFS))
        pp = ctx.enter_context(tc.tile_pool(name="pp", bufs=8, space="PSUM"))

        a_eng = getattr(nc, A_ENGINE)
        APACK = 8                                    # k-chunks per A tile

        def load_a(g):
            # [128, APACK, M_LOC] : partition p holds k-rows g*APACK*128
            # + j*128 + p for j in 0..APACK
            a_t = apool.tile([128, APACK * M_LOC], io_dt, name=f"a_{g}",
                             tag="a")
            src = a_d.ap()[g * APACK * 128:(g + 1) * APACK * 128, :] \
                .rearrange("(j p) m -> p j m", p=128)
            a_eng.dma_start(a_t[:], src)
            return a_t

        a_tiles = [load_a(g) for g in range(KC // APACK)]

        def lhsT_slice(kc, mrow):
            # stationary [128k x 128m] at k-chunk kc, m-rows mrow..mrow+128
            ga, ja = divmod(kc, APACK)
            off = ja * M_LOC + mrow
            return a_tiles[ga][:, off:off + 128]

        for rep in range(REPS):
            for mbp in range(NSUB):              # m-passes (B re-read each)
                for nbi in range(NBI):
                    psums = [[pp.tile([128, 512], f32,
                                      name=f"p_{rep}_{mbp}_{nbi}_{ms}_{ns}",
                                      tag="p")
                              for ns in range(NSUB)]
                             for ms in range(MSP)]
                    for g in range(KG):
                        if FAKE_B and not (rep == 0 and mbp == 0
                                           and nbi == 0 and g == 0):
                            b_t = fake_bt
                        else:
                            b_t = bpool.tile([128, KPACK * NBW], io_dt,
                                             name=f"b_{rep}_{mbp}_{nbi}_{g}",
                                             tag="b")
                            src = b_d.ap()[g * KPACK * 128:
                                           (g + 1) * KPACK * 128,
                                           nbi * NBW:(nbi + 1) * NBW] \
                                .rearrange("(j p) n -> p j n", p=128)
                            eng = nc.scalar if (SPLIT and g % 2) else nc.sync
                            eng.dma_start(b_t[:], src)
                            if FAKE_B:
                                fake_bt = b_t
                        for j in range(KPACK):
                            kc = g * KPACK + j
                            for ms in range(MSP):
                                lhsT = lhsT_slice(
                                    kc, mbp * MSP * 128 + ms * 128)
                                for ns in range(NSUB):
                                    rhs = b_t[:, j * NBW + ns * 512:
                                              j * NBW + (ns + 1) * 512]
                                    nc.tensor.matmul(psums[ms][ns][:], lhsT,
                                                     rhs, start=(kc == 0),
                                                     stop=(kc == KC - 1))
                    if NO_DRAIN and not (rep == REPS - 1 and mbp == NSUB - 1
                                         and nbi == NBI - 1):
                        continue
                    c_t = cpool.tile([128, MSP * NSUB * 512], out_dt,
                                     name=f"c_{rep}_{mbp}_{nbi}", tag="c")
                    for ms in range(MSP):
                        for ns in range(NSUB):
                            sl = (ms * NSUB + ns) * 512
                            dst_sl = c_t[:, sl:sl + 512]
                            if DRAIN == "both" and (ms * NSUB + ns) % 2:
                                nc.scalar.activation(
                                    dst_sl, psums[ms][ns][:],
                                    mybir.ActivationFunctionType.Copy)
                            else:
                                nc.vector.tensor_copy(dst_sl, psums[ms][ns][:])
                    rows = mbp * MSP * 128
                    if NSUB == 1:
                        dst = o_d.ap()[:, nbi * 512:(nbi + 1) * 512] \
                            .rearrange("(ms p) n -> p ms n", p=128)
                    else:
                        dst = o_d.ap()[rows:rows + MSP * 128,
                                       nbi * NBW:(nbi + 1) * NBW] \
                            .rearrange("(ms p) (ns n) -> p ms ns n",
                                       p=128, ns=NSUB)
                    getattr(nc, STORE_ENGINE).dma_start(dst, c_t[:])

    nc.compile()
    return nc


def get_nc(dt_name=None):
    dt_name = dt_name or DTYPE
    if dt_name not in _NC_CACHE:
        _NC_CACHE[dt_name] = _build_nc(dt_name)
    return _NC_CACHE[dt_name]


def make_in_maps(input, weight, dt_name=None):
    """Host-side shard + layout prep. Returns in_maps for cores 0..7."""
    dt_name = dt_name or DTYPE
    input = np.asarray(input, dtype=np.float32)
    weight = np.asarray(weight, dtype=np.float32)
    assert input.shape == (WS, M, LK), input.shape
    assert weight.shape == (WS, N, LK), weight.shape

    if dt_name == "bfloat16":
        import ml_dtypes
        np_dt = ml_dtypes.bfloat16
    else:
        np_dt = np.float32

    # B[r*LK + k, n] = weight[r, n, k]  -> [K, N]
    b_full = np.ascontiguousarray(
        weight.transpose(0, 2, 1).reshape(K, N).astype(np_dt))
    in_maps = []
    for c in range(N_CORES):
        # A_c[r*LK + k, m] = input[r, c*M_LOC + m, k]  -> [K, M_LOC]
        a_c = np.ascontiguousarray(
            input[:, c * M_LOC:(c + 1) * M_LOC, :]
            .transpose(0, 2, 1).reshape(K, M_LOC).astype(np_dt))
        in_maps.append({"a": a_c, "b": b_full})
    return in_maps


def kernel(input, weight):
    from concourse import bass_utils

    nc = get_nc()
    in_maps = make_in_maps(input, weight)
    res = bass_utils.run_bass_kernel_spmd(
        nc, in_maps, core_ids=list(range(N_CORES)))
    out = np.stack([res.results[c]["o"] for c in range(N_CORES)], axis=0)
    return out.astype(np.float32)


# revision 16
# speedup vs baseline: 4.8831x; 1.0532x over previous
"""GEMM + reduce-scatter (nn_GemmRSIntraNode) as a Bass/Tile kernel on 8 trn2 cores.

Full semantics: out = einsum('rmk,rnk->mn', input, weight).reshape(8, 1024, 4096)
with input [8, 8192, 1024] f32 and weight [8, 4096, 1024] f32.

Sharding choice: each core c directly computes output rows
[c*1024:(c+1)*1024] of the reduced result — a [1024, 8192] x [8192, 4096]
GEMM per core where the contraction axis is (rank, local_k) flattened.
The rank-sum IS the K-axis contraction, so no cross-core communication is
needed; the "reduce-scatter" is absorbed into the GEMM. Inputs are
pre-transposed host-side into [K, M] / [K, N] layouts (bf16) so the
device kernel does only contiguous DMA and matmuls.

Perf structure (v2): IO in bf16 (measured norm_rel 2.9e-3, gate is
2e-2; bf16 matmul runs at the same 1 cyc/row PE rate as f32r but
halves all HBM traffic). The whole per-core A panel [K, 1024] bf16 =
16 MB stays resident in SBUF, so B [K, 4096] bf16 = 64 MB streams
through exactly once; per-core traffic is ~88 MB vs the ~304 MB of the
f32r v1. DMAs are packed KPACK k-chunks at a time (1 MB per dma_start,
[128, 8, 512] DRAM APs) to amortize per-DMA completion latency. B
loads keep the sync HWDGE ring to themselves; A loads ride the SWDGE
(gpsimd) ring and output stores the scalar HWDGE ring, so a store's
wait-for-drain never blocks issuing the next block's B loads (HWDGE is
FIFO per issuing engine). PSUM->SBUF drains alternate DVE/ACT so the
bank the next n-block's first matmuls need frees ~2x sooner.

Measured on hw (REPS-slope, i.e. marginal NEFF body time per GEMM):
~1.05-1.15 ms vs 874 us pure-matmul floor (4096 N=512 matmuls at the
warm 2.4 GHz rate; verified sustained at 214 ns/MM in isolation).
Wall-clock per-exec through the axon-relay PJRT path adds a fixed
~1.1 ms/exec pipeline stage that is independent of the kernel (an
empty kernel measures the same), which is why test.py reports the
slope rather than the batch-amortized wall clock.
"""

import os
from contextlib import ExitStack

import numpy as np

WS = 8
M = 8192
N = 4096
LK = 1024
K = WS * LK          # 8192 contraction (rank*local_k)
M_LOC = M // WS      # 1024 output rows per core
N_CORES = 8

DTYPE = os.environ.get("BASS_KERNEL_DTYPE", "bfloat16")

_NC_CACHE = {}


def _build_nc(dt_name):
    import concourse.tile as tile
    from concourse import bacc, mybir

    f32 = mybir.dt.float32
    if dt_name == "bfloat16":
        io_dt = mybir.dt.bfloat16
    elif dt_name == "float32r":
        io_dt = mybir.dt.float32r
    else:
        raise ValueError(dt_name)

    KC = K // 128                                  # 64 k-chunks of 128
    # NSUB: n-subtiles sharing one stationary operand (LDWEIGHTS amortization)
    NSUB = int(os.environ.get("BASS_NSUB", "1"))
    NBW = 512 * NSUB                               # n-block width
    NBI = N // NBW                                 # n-blocks
    MSP = (M_LOC // 128) // NSUB                   # m-subtiles per pass
    KPACK = int(os.environ.get("BASS_KPACK", str(8 // NSUB)))
    KG = KC // KPACK                               # dma groups along K
    NB = N // 512                                  # 8 n-blocks
    MSUB = M_LOC // 128                            # 8 psum row-tiles
    B_BUFS = int(os.environ.get("BASS_B_BUFS", "4"))
    C_BUFS = int(os.environ.get("BASS_C_BUFS", "2"))
    SPLIT = os.environ.get("BASS_SPLIT", "0") == "1"   # B loads on 2 rings
    A_ENGINE = os.environ.get("BASS_A_ENGINE", "gpsimd")
    # drain: "both" splits psum->sbuf copies across DVE+ACT so the bank the
    # next n-block's first matmuls need frees ~2x sooner; "dve" uses DVE only
    DRAIN = os.environ.get("BASS_DRAIN", "both")
    OUT_BF16 = os.environ.get("BASS_OUT_BF16", "1") == "1"
    # engine for output stores; "scalar" keeps the sync ring pure B-loads so
    # the store's wait-for-drain never blocks issuing the next block's loads
    STORE_ENGINE = os.environ.get("BASS_STORE_ENGINE", "scalar")
    # surgical-isolation knobs (experiments only; wrong results when set)
    NO_DRAIN = os.environ.get("BASS_NO_DRAIN", "0") == "1"
    FAKE_B = os.environ.get("BASS_FAKE_B", "0") == "1"
    REPS = int(os.environ.get("BASS_REPS", "1"))
    assert MSUB <= 8                               # one PSUM bank per row-tile

    nc = bacc.Bacc("TRN2", target_bir_lowering=False, debug=False,
                   num_devices=N_CORES)
    out_dt = mybir.dt.bfloat16 if OUT_BF16 else f32
    a_d = nc.dram_tensor("a", [K, M_LOC], io_dt, kind="ExternalInput")
    b_d = nc.dram_tensor("b", [K, N], io_dt, kind="ExternalInput")
    o_d = nc.dram_tensor("o", [M_LOC, N], out_dt, kind="ExternalOutput")

    with tile.TileContext(nc) as tc, ExitStack() as ctx:
        apool = ctx.enter_context(tc.tile_pool(name="apool", bufs=KC // 8))
        bpool = ctx.enter_context(tc.tile_pool(name="bpool", bufs=B_BUFS))
        cpool = ctx.enter_context(tc.tile_pool(name="cpool", bufs=C_BUFS))
        pp = ctx.enter_context(tc.tile_pool(name="pp", bufs=8, space="PSUM"))

        a_eng = getattr(nc, A_ENGINE)
        APACK = 8                                    # k-chunks per A tile

        def load_a(g):
            # [128, APACK, M_LOC] : partition p holds k-rows g*APACK*128
            # + j*128 + p for j in 0..APACK
            a_t = apool.tile([128, APACK * M_LOC], io_dt, name=f"a_{g}",
                             tag="a")
            src = a_d.ap()[g * APACK * 128:(g + 1) * APACK * 128, :] \
                .rearrange("(j p) m -> p j m", p=128)
            a_eng.dma_start(a_t[:], src)
            return a_t

        a_tiles = [load_a(g) for g in range(KC // APACK)]

        def lhsT_slice(kc, mrow):
            # stationary [128k x 128m] at k-chunk kc, m-rows mrow..mrow+128
            ga, ja = divmod(kc, APACK)
            off = ja * M_LOC + mrow
            return a_tiles[ga][:, off:off + 128]

        for rep in range(REPS):
            for mbp in range(NSUB):              # m-passes (B re-read each)
                for nbi in range(NBI):
                    psums = [[pp.tile([128, 512], f32,
                                      name=f"p_{rep}_{mbp}_{nbi}_{ms}_{ns}",
                                      tag="p")
                              for ns in range(NSUB)]
                             for ms in range(MSP)]
                    for g in range(KG):
                        if FAKE_B and not (rep == 0 and mbp == 0
                                           and nbi == 0 and g == 0):
                            b_t = fake_bt
                        else:
                            b_t = bpool.tile([128, KPACK * NBW], io_dt,
                                             name=f"b_{rep}_{mbp}_{nbi}_{g}",
                                             tag="b")
                            src = b_d.ap()[g * KPACK * 128:
                                           (g + 1) * KPACK * 128,
                                           nbi * NBW:(nbi + 1) * NBW] \
                                .rearrange("(j p) n -> p j n", p=128)
                            eng = nc.scalar if (SPLIT and g % 2) else nc.sync
                            eng.dma_start(b_t[:], src)
                            if FAKE_B:
                                fake_bt = b_t
                        for j in range(KPACK):
                            kc = g * KPACK + j
                            for ms in range(MSP):
                                lhsT = lhsT_slice(
                                    kc, mbp * MSP * 128 + ms * 128)
                                for ns in range(NSUB):
                                    rhs = b_t[:, j * NBW + ns * 512:
                                              j * NBW + (ns + 1) * 512]
                                    nc.tensor.matmul(psums[ms][ns][:], lhsT,
                                                     rhs, start=(kc == 0),
                                                     stop=(kc == KC - 1))
                    if NO_DRAIN and not (rep == REPS - 1 and mbp == NSUB - 1
                                         and nbi == NBI - 1):
                        continue
                    c_t = cpool.tile([128, MSP * NSUB * 512], out_dt,
                                     name=f"c_{rep}_{mbp}_{nbi}", tag="c")
                    for ms in range(MSP):
                        for ns in range(NSUB):
                            sl = (ms * NSUB + ns) * 512
                            dst_sl = c_t[:, sl:sl + 512]
                            if DRAIN == "both" and (ms * NSUB + ns) % 2:
                                nc.scalar.activation(
                                    dst_sl, psums[ms][ns][:],
                                    mybir.ActivationFunctionType.Copy)
                            else:
                                nc.vector.tensor_copy(dst_sl, psums[ms][ns][:])
                    rows = mbp * MSP * 128
                    if NSUB == 1:
                        dst = o_d.ap()[:, nbi * 512:(nbi + 1) * 512] \
                            .rearrange("(ms p) n -> p ms n", p=128)
                    else:
                        dst = o_d.ap()[rows:rows + MSP * 128,
                                       nbi * NBW:(nbi + 1) * NBW] \
                            .rearrange("(ms p) (ns n) -> p ms ns n",
                                       p=128, ns=NSUB)
                    getattr(nc, STORE_ENGINE).dma_start(dst, c_t[:])

    nc.compile()
    return nc


def get_nc(dt_name=None):
    dt_name = dt_name or DTYPE
    if dt_name not in _NC_CACHE:
        _NC_CACHE[dt_name] = _build_nc(dt_name)
    return _NC_CACHE[dt_name]


def make_in_maps(input, weight, dt_name=None):
    """Host-side shard + layout prep. Returns in_maps for cores 0..7."""
    dt_name = dt_name or DTYPE
    input = np.asarray(input, dtype=np.float32)
    weight = np.asarray(weight, dtype=np.float32)
    assert input.shape == (WS, M, LK), input.shape
    assert weight.shape == (WS, N, LK), weight.shape

    if dt_name == "bfloat16":
        import ml_dtypes
        np_dt = ml_dtypes.bfloat16
    else:
        np_dt = np.float32

    # B[r*LK + k, n] = weight[r, n, k]  -> [K, N]
    b_full = np.ascontiguousarray(
        weight.transpose(0, 2, 1).reshape(K, N).astype(np_dt))
    in_maps = []
    for c in range(N_CORES):
        # A_c[r*LK + k, m] = input[r, c*M_LOC + m, k]  -> [K, M_LOC]
        a_c = np.ascontiguousarray(
            input[:, c * M_LOC:(c + 1) * M_LOC, :]
            .transpose(0, 2, 1).reshape(K, M_LOC).astype(np_dt))
        in_maps.append({"a": a_c, "b": b_full})
    return in_maps


def kernel(input, weight):
    from concourse import bass_utils

    nc = get_nc()
    in_maps = make_in_maps(input, weight)
    res = bass_utils.run_bass_kernel_spmd(
        nc, in_maps, core_ids=list(range(N_CORES)))
    out = np.stack([res.results[c]["o"] for c in range(N_CORES)], axis=0)
    return out.astype(np.float32)


# revision 18
# speedup vs baseline: 4.9071x; 1.0049x over previous
"""GEMM + reduce-scatter (nn_GemmRSIntraNode) as a Bass/Tile kernel on 8 trn2 cores.

Full semantics: out = einsum('rmk,rnk->mn', input, weight).reshape(8, 1024, 4096)
with input [8, 8192, 1024] f32 and weight [8, 4096, 1024] f32.

Sharding choice: each core c directly computes output rows
[c*1024:(c+1)*1024] of the reduced result — a [1024, 8192] x [8192, 4096]
GEMM per core where the contraction axis is (rank, local_k) flattened.
The rank-sum IS the K-axis contraction, so no cross-core communication is
needed; the "reduce-scatter" is absorbed into the GEMM. Inputs are
pre-transposed host-side into [K, M] / [K, N] layouts (bf16) so the
device kernel does only contiguous DMA and matmuls.

Perf structure (v2): IO in bf16 (measured norm_rel 2.9e-3, gate is
2e-2; bf16 matmul runs at the same 1 cyc/row PE rate as f32r but
halves all HBM traffic). The whole per-core A panel [K, 1024] bf16 =
16 MB stays resident in SBUF, so B [K, 4096] bf16 = 64 MB streams
through exactly once; per-core traffic is ~88 MB vs the ~304 MB of the
f32r v1. DMAs are packed KPACK k-chunks at a time (1 MB per dma_start,
[128, 8, 512] DRAM APs) to amortize per-DMA completion latency. B
loads keep the sync HWDGE ring to themselves; A loads ride the SWDGE
(gpsimd) ring and output stores the scalar HWDGE ring, so a store's
wait-for-drain never blocks issuing the next block's B loads (HWDGE is
FIFO per issuing engine). PSUM->SBUF drains alternate DVE/ACT so the
bank the next n-block's first matmuls need frees ~2x sooner.

Measured on hw (REPS-slope, i.e. marginal NEFF body time per GEMM):
~1.05-1.15 ms vs 874 us pure-matmul floor (4096 N=512 matmuls at the
warm 2.4 GHz rate; verified sustained at 214 ns/MM in isolation).
Wall-clock per-exec through the axon-relay PJRT path adds a fixed
~1.1 ms/exec pipeline stage that is independent of the kernel (an
empty kernel measures the same), which is why test.py reports the
slope rather than the batch-amortized wall clock.
"""

import os
from contextlib import ExitStack

import numpy as np

WS = 8
M = 8192
N = 4096
LK = 1024
K = WS * LK          # 8192 contraction (rank*local_k)
M_LOC = M // WS      # 1024 output rows per core
N_CORES = 8

DTYPE = os.environ.get("BASS_KERNEL_DTYPE", "bfloat16")

_NC_CACHE = {}


def _build_nc(dt_name):
    import concourse.tile as tile
    from concourse import bacc, mybir

    f32 = mybir.dt.float32
    if dt_name == "bfloat16":
        io_dt = mybir.dt.bfloat16
    elif dt_name == "float32r":
        io_dt = mybir.dt.float32r
    else:
        raise ValueError(dt_name)

    KC = K // 128                                  # 64 k-chunks of 128
    # NSUB: n-subtiles sharing one stationary operand (LDWEIGHTS amortization)
    NSUB = int(os.environ.get("BASS_NSUB", "1"))
    NBW = 512 * NSUB                               # n-block width
    NBI = N // NBW                                 # n-blocks
    MSP = (M_LOC // 128) // NSUB                   # m-subtiles per pass
    KPACK = int(os.environ.get("BASS_KPACK", str(8 // NSUB)))
    KG = KC // KPACK                               # dma groups along K
    NB = N // 512                                  # 8 n-blocks
    MSUB = M_LOC // 128                            # 8 psum row-tiles
    B_BUFS = int(os.environ.get("BASS_B_BUFS", "4"))
    C_BUFS = int(os.environ.get("BASS_C_BUFS", "2"))
    SPLIT = os.environ.get("BASS_SPLIT", "0") == "1"   # B loads on 2 rings
    A_ENGINE = os.environ.get("BASS_A_ENGINE", "gpsimd")
    # drain: "both" splits psum->sbuf copies across DVE+ACT so the bank the
    # next n-block's first matmuls need frees ~2x sooner; "dve" uses DVE only
    DRAIN = os.environ.get("BASS_DRAIN", "both")
    OUT_BF16 = os.environ.get("BASS_OUT_BF16", "1") == "1"
    # engine for output stores; "scalar" keeps the sync ring pure B-loads so
    # the store's wait-for-drain never blocks issuing the next block's loads
    STORE_ENGINE = os.environ.get("BASS_STORE_ENGINE", "scalar")
    # surgical-isolation knobs (experiments only; wrong results when set)
    NO_DRAIN = os.environ.get("BASS_NO_DRAIN", "0") == "1"
    FAKE_B = os.environ.get("BASS_FAKE_B", "0") == "1"
    REPS = int(os.environ.get("BASS_REPS", "1"))
    assert MSUB <= 8                               # one PSUM bank per row-tile

    nc = bacc.Bacc("TRN2", target_bir_lowering=False, debug=False,
                   num_devices=N_CORES)
    out_dt = mybir.dt.bfloat16 if OUT_BF16 else f32
    a_d = nc.dram_tensor("a", [K, M_LOC], io_dt, kind="ExternalInput")
    b_d = nc.dram_tensor("b", [K, N], io_dt, kind="ExternalInput")
    o_d = nc.dram_tensor("o", [M_LOC, N], out_dt, kind="ExternalOutput")

    with tile.TileContext(nc) as tc, ExitStack() as ctx:
        apool = ctx.enter_context(tc.tile_pool(
            name="apool",
            bufs=1 if os.environ.get("BASS_ABIG", "0") == "1" else KC // 8))
        bpool = ctx.enter_context(tc.tile_pool(name="bpool", bufs=B_BUFS))
        cpool = ctx.enter_context(tc.tile_pool(name="cpool", bufs=C_BUFS))
        pp = ctx.enter_context(tc.tile_pool(name="pp", bufs=8, space="PSUM"))

        a_eng = getattr(nc, A_ENGINE)
        APACK = 8                                    # k-chunks per A DMA
        ABIG = os.environ.get("BASS_ABIG", "0") == "1"

        if ABIG:
            # single 128 KB/partition allocation: lhsT never changes source
            # tile, only offsets (probes the tile-change penalty)
            a_big = apool.tile([128, KC * M_LOC], io_dt, name="a_big",
                               tag="a")
            for g in range(KC // APACK):
                src = a_d.ap()[g * APACK * 128:(g + 1) * APACK * 128, :] \
                    .rearrange("(j p) m -> p j m", p=128)
                a_eng.dma_start(
                    a_big[:, g * APACK * M_LOC:(g + 1) * APACK * M_LOC], src)

            def lhsT_slice(kc, mrow):
                off = kc * M_LOC + mrow
                return a_big[:, off:off + 128]
        else:
            def load_a(g):
                # [128, APACK, M_LOC] : partition p holds k-rows g*APACK*128
                # + j*128 + p for j in 0..APACK
                a_t = apool.tile([128, APACK * M_LOC], io_dt, name=f"a_{g}",
                                 tag="a")
                src = a_d.ap()[g * APACK * 128:(g + 1) * APACK * 128, :] \
                    .rearrange("(j p) m -> p j m", p=128)
                a_eng.dma_start(a_t[:], src)
                return a_t

            a_tiles = [load_a(g) for g in range(KC // APACK)]

            def lhsT_slice(kc, mrow):
                # stationary [128k x 128m] at k-chunk kc, m-rows mrow..+128
                ga, ja = divmod(kc, APACK)
                off = ja * M_LOC + mrow
                return a_tiles[ga][:, off:off + 128]

        for rep in range(REPS):
            for mbp in range(NSUB):              # m-passes (B re-read each)
                for nbi in range(NBI):
                    psums = [[pp.tile([128, 512], f32,
                                      name=f"p_{rep}_{mbp}_{nbi}_{ms}_{ns}",
                                      tag="p")
                              for ns in range(NSUB)]
                             for ms in range(MSP)]
                    for g in range(KG):
                        if FAKE_B and not (rep == 0 and mbp == 0
                                           and nbi == 0 and g == 0):
                            b_t = fake_bt
                        else:
                            b_t = bpool.tile([128, KPACK * NBW], io_dt,
                                             name=f"b_{rep}_{mbp}_{nbi}_{g}",
                                             tag="b")
                            src = b_d.ap()[g * KPACK * 128:
                                           (g + 1) * KPACK * 128,
                                           nbi * NBW:(nbi + 1) * NBW] \
                                .rearrange("(j p) n -> p j n", p=128)
                            eng = nc.scalar if (SPLIT and g % 2) else nc.sync
                            eng.dma_start(b_t[:], src)
                            if FAKE_B:
                                fake_bt = b_t
                        for j in range(KPACK):
                            kc = g * KPACK + j
                            for ms in range(MSP):
                                lhsT = lhsT_slice(
                                    kc, mbp * MSP * 128 + ms * 128)
                                for ns in range(NSUB):
                                    rhs = b_t[:, j * NBW + ns * 512:
                                              j * NBW + (ns + 1) * 512]
                                    nc.tensor.matmul(psums[ms][ns][:], lhsT,
                                                     rhs, start=(kc == 0),
                                                     stop=(kc == KC - 1))
                    if NO_DRAIN and not (rep == REPS - 1 and mbp == NSUB - 1
                                         and nbi == NBI - 1):
                        continue
                    c_t = cpool.tile([128, MSP * NSUB * 512], out_dt,
                                     name=f"c_{rep}_{mbp}_{nbi}", tag="c")
                    for ms in range(MSP):
                        for ns in range(NSUB):
                            sl = (ms * NSUB + ns) * 512
                            dst_sl = c_t[:, sl:sl + 512]
                            if DRAIN == "both" and (ms * NSUB + ns) % 2:
                                nc.scalar.activation(
                                    dst_sl, psums[ms][ns][:],
                                    mybir.ActivationFunctionType.Copy)
                            else:
                                nc.vector.tensor_copy(dst_sl, psums[ms][ns][:])
                    rows = mbp * MSP * 128
                    if NSUB == 1:
                        dst = o_d.ap()[:, nbi * 512:(nbi + 1) * 512] \
                            .rearrange("(ms p) n -> p ms n", p=128)
                    else:
                        dst = o_d.ap()[rows:rows + MSP * 128,
                                       nbi * NBW:(nbi + 1) * NBW] \
                            .rearrange("(ms p) (ns n) -> p ms ns n",
                                       p=128, ns=NSUB)
                    getattr(nc, STORE_ENGINE).dma_start(dst, c_t[:])

    nc.compile()
    return nc


def get_nc(dt_name=None):
    dt_name = dt_name or DTYPE
    if dt_name not in _NC_CACHE:
        _NC_CACHE[dt_name] = _build_nc(dt_name)
    return _NC_CACHE[dt_name]


def make_in_maps(input, weight, dt_name=None):
    """Host-side shard + layout prep. Returns in_maps for cores 0..7."""
    dt_name = dt_name or DTYPE
    input = np.asarray(input, dtype=np.float32)
    weight = np.asarray(weight, dtype=np.float32)
    assert input.shape == (WS, M, LK), input.shape
    assert weight.shape == (WS, N, LK), weight.shape

    if dt_name == "bfloat16":
        import ml_dtypes
        np_dt = ml_dtypes.bfloat16
    else:
        np_dt = np.float32

    # B[r*LK + k, n] = weight[r, n, k]  -> [K, N]
    b_full = np.ascontiguousarray(
        weight.transpose(0, 2, 1).reshape(K, N).astype(np_dt))
    in_maps = []
    for c in range(N_CORES):
        # A_c[r*LK + k, m] = input[r, c*M_LOC + m, k]  -> [K, M_LOC]
        a_c = np.ascontiguousarray(
            input[:, c * M_LOC:(c + 1) * M_LOC, :]
            .transpose(0, 2, 1).reshape(K, M_LOC).astype(np_dt))
        in_maps.append({"a": a_c, "b": b_full})
    return in_maps


def kernel(input, weight):
    from concourse import bass_utils

    nc = get_nc()
    in_maps = make_in_maps(input, weight)
    res = bass_utils.run_bass_kernel_spmd(
        nc, in_maps, core_ids=list(range(N_CORES)))
    out = np.stack([res.results[c]["o"] for c in range(N_CORES)], axis=0)
    return out.astype(np.float32)
